# revision 28
# baseline (speedup 1.0000x reference)
"""BirthDeathAttention kernel for 8 Trainium2 NeuronCores.

Math note: in the reference, both `persistence_bias` ([1,H,1,1]) and
`importance_weights[:, None, :, None] * 0.1` ([B,1,N,1]) are constant along
the softmax (key) axis, so they cancel exactly inside the softmax.  The
module is therefore plain multi-head attention + output projection.

Sharding (per the tensor-parallel hint): core = (batch b, head-group g),
b in {0,1}, g in {0..3}, each core handling 4 of the 16 heads for one batch
element.  Each core computes a partial output projection (its heads' slice
of W_proj rows); the host sums the 4 partials per batch and adds b_proj.

Schedule: wave-pipelined, ACT-paced.  Wave w = (block w//16, key-tile w%16):
  S-pair(w):  two row-tiled concurrent matmuls -> one [128,1024] PSUM tile
  exp(w):     one ACTIVATE [128,1024] PSUM->SBUF bf16 (~1.15us, the pacer)
  U(w):       two matmuls (M=65: v|ones emits the softmax denominator as
              row 64) accumulating into the block's U pair; blocks
              alternate (s0,s1)/(s2,s3)
The q/k/v chains drain from a deadline-checked FIFO interleaved so each
chain's copy_out is emitted at least one cycle before its first consumer:
k-pair0 chains by their S deadlines, v-chains early (U consumes them from
U_T0), pair-1 chains on the s2/s3 slots until U claims them at block 1.
U starts at cycle U_T0 at 1 wave/cycle (PE stays under the exp pace),
catches up at 1.75 waves/cycle once the chains drain, then trails exp;
U waves land on even cycles in 2-cycle bursts to halve the S<->U
weight-switch tax on the in-order PE.  The output projection runs in the
tail: pieces rotate over four accumulator homes (psS halves + both psU
pairs) so four are in flight, copies balance scalar/vector, stores issue
from the idle gpsimd queue, and mt 0-11 run concurrently with the last
block's normalization (whose reciprocal broadcast uses a PE matmul
against ones instead of the DRAM round-trip).

PSUM (8 banks): psS 2x[128,1024] (4) + psU slots s0..s3 (4).
"""

import sys

if "/opt/trn_rl_repo" not in sys.path:
    sys.path.insert(0, "/opt/trn_rl_repo")

from collections import deque

import numpy as np
import ml_dtypes

import concourse.bass as bass
import concourse.mybir as mybir
import concourse.tile as tile
from concourse.bass_utils import run_bass_kernel_spmd

DIM = 1024
N = 2048
B = 2
HEADS = 16
HEAD_DIM = 64
SCALE = HEAD_DIM ** -0.5
HPG = 4          # heads per group (per core)
GC = HPG * HEAD_DIM  # channels per core = 256
BF16 = mybir.dt.bfloat16
F32 = mybir.dt.float32

KT = DIM // 128   # 8 contraction tiles over model dim
# wqk column blocks: [k-pair0 | q-pair0 | k-pair1 | q-pair1]
CT_COL = {2: 0, 0: 1, 3: 2, 1: 3}  # chain ct -> wqk column block
NKT = N // 128    # 16 key tiles per block
NW = 8 * NKT      # 128 waves
BLOCKS = [(0, 0), (1, 0), (2, 0), (3, 0), (0, 1), (1, 1), (2, 1), (3, 1)]
U_T0 = 38         # first cycle with U matmuls (1 wave/cycle from here)
U_CATCH = 54      # chains drained; catch up at U_CRATE waves/cycle
U_CRATE = 1.75
A_NS = 215        # PE cost of one qk-chain matmul ([128,512])
B_NS = 180        # PE cost of one v-chain matmul ([128,256] + issue dead)


def _build_u_sched():
    """U emission cycles.  Waves are batched into 4-cycle bursts (2-cycle
    near the end) so the PE pays the S<->U weight-switch tax once per
    burst instead of every cycle."""
    sched = {}
    c = float(U_T0)
    for w in range(NW):
        raw = max(w + 3, int(c))
        step = 4 if w < 96 else 2
        sched[w] = raw + (-raw) % step
        c += 1.0 if c < U_CATCH else 1.0 / U_CRATE
    return sched


U_SCHED = _build_u_sched()
EPOOL = max((c + 1) - sum(1 for w in range(NW) if U_SCHED[w] <= c)
            for c in range(NW + 8)) + 4


def _split_multi_waits(nc, max_waits=1):
    """The walrus build in this container accepts at most one sync-wait per
    instruction.  Hoist extra waits onto single-wait NOPs inserted just
    before the instruction in its engine's program order."""
    uid = [0]
    for f in nc.m.functions:
        for bb in f.blocks:
            insts = bb.instructions
            new = []
            changed = False
            for inst in insts:
                si = inst.sync_info
                if si is not None and len(si.on_wait) > max_waits:
                    waits = list(si.on_wait)
                    for w in waits[:-max_waits]:
                        nop = mybir.InstNoOp(
                            name=f"I-splitw-{uid[0]}", ins=[], outs=[])
                        uid[0] += 1
                        nop.engine = inst.engine
                        nop.sync_info = mybir.SyncInfo(
                            on_wait=[w], on_update=[])
                        new.append(nop)
                    si.on_wait = waits[-max_waits:]
                    inst.sync_info = si
                    changed = True
                new.append(inst)
            if changed:
                bb.instructions = new


def build_core_kernel() -> bass.Bass:
    nc = bass.Bass()
    xT = nc.declare_dram_parameter("xT", [DIM, N], BF16, isOutput=False)
    wqk = nc.declare_dram_parameter("wqk", [DIM, 2 * GC], BF16, isOutput=False)
    wv = nc.declare_dram_parameter("wv", [DIM, GC], BF16, isOutput=False)
    wp = nc.declare_dram_parameter("wp", [GC, DIM], BF16, isOutput=False)
    out = nc.declare_dram_parameter("out", [N, DIM], BF16, isOutput=True)

    xT_r = xT.rearrange("(kt p) n -> p kt n", p=128)
    wqk_r = wqk.rearrange("(kt p) c -> p kt c", p=128)
    wv_r = wv.rearrange("(kt p) c -> p kt c", p=128)
    wp_r = wp.rearrange("(pair p) c -> p pair c", p=128)

    with tile.TileContext(nc) as tc:
        from contextlib import ExitStack

        with ExitStack() as ctx:
            consts = ctx.enter_context(tc.tile_pool(name="consts", bufs=1))
            sbuf = ctx.enter_context(tc.tile_pool(name="sbuf", bufs=1))
            epool = ctx.enter_context(tc.tile_pool(name="epool", bufs=EPOOL))
            npool = ctx.enter_context(tc.tile_pool(name="npool", bufs=2))
            rdram = ctx.enter_context(
                tc.tile_pool(name="rdram", bufs=2, space="DRAM"))
            opool = ctx.enter_context(tc.tile_pool(name="opool", bufs=4))
            psS = ctx.enter_context(
                tc.tile_pool(name="psS", bufs=2, space="PSUM"))
            psU = ctx.enter_context(
                tc.tile_pool(name="psU", bufs=1, space="PSUM"))

            # --- resident SBUF tensors -------------------------------------
            xT_sb = sbuf.tile([128, KT, N], BF16, tag="xT")
            wqk_sb = consts.tile([128, KT, 2 * GC], BF16, tag="wqk")
            wv_sb = consts.tile([128, KT, GC], BF16, tag="wv")
            wp_sb = consts.tile([128, 2, DIM], BF16, tag="wp")
            ones_sb = consts.tile([128, 64], BF16, tag="ones")
            warm_sb = consts.tile([128, 512], BF16, tag="warm")
            qk_sb = sbuf.tile([128, 4, N], BF16, tag="qk")
            # v with a ones column appended per head ([v_h | 1], stride 65):
            # the ones column turns attention@v into a matmul that also
            # emits the softmax denominator as output row 64
            v_sb = sbuf.tile([128, NKT, HPG * 65], BF16, tag="v")
            o_sb = sbuf.tile([128, 2, N], BF16, tag="o")

            nc.vector.memset(ones_sb[:], 1.0)
            nc.vector.memset(warm_sb[:], 1.0)
            v_view = v_sb.rearrange("p nt (h c) -> p nt h c", c=65)
            nc.vector.memset(v_view[:, :, :, 64:65], 1.0)

            # --- input DMAs.  Critical-path first and contention-ordered:
            # the k0/q0 weight half (scalar, one 512B-segment DMA) and the
            # first xT token-block (sync, 2-ktile chunks) feed the inline
            # prologue chains; gpsimd streams wv (v-chains need it next),
            # then the pair-1 weights, then wp (needed last).
            # ALL input DMAs on one queue: per-queue transfers run in
            # scheduled order, so arrival order (critical-path first) is
            # controllable — split queues share HBM unpredictably and
            # starve the small critical transfers.
            nc.sync.dma_start(out=wqk_sb[:, :, 0:256],
                              in_=wqk_r[:, :, 0:256])
            nc.sync.dma_start(out=xT_sb[:, :, 0:512], in_=xT_r[:, :, 0:512])
            nc.sync.dma_start(out=xT_sb[:, :, 512:1024],
                              in_=xT_r[:, :, 512:1024])
            nc.sync.dma_start(out=wv_sb[:], in_=wv_r[:])
            nc.sync.dma_start(out=xT_sb[:, :, 1024:1536],
                              in_=xT_r[:, :, 1024:1536])
            nc.sync.dma_start(out=wqk_sb[:, :, 256:512],
                              in_=wqk_r[:, :, 256:512])
            nc.sync.dma_start(out=xT_sb[:, :, 1536:2048],
                              in_=xT_r[:, :, 1536:2048])
            nc.sync.dma_start(out=wp_sb[:], in_=wp_r[:])

            # --- wave emitters --------------------------------------------
            def s_pair(w):
                nqb, pair = BLOCKS[w // NKT]
                nkt = w % NKT
                qt = qk_sb[:, pair, :]
                kt_sb = qk_sb[:, 2 + pair, :]
                st = psS.tile([128, 1024], F32, tag="st")
                for hh in range(2):
                    nc.tensor.matmul(
                        st[:, hh * 512:(hh + 1) * 512],
                        lhsT=kt_sb[hh * 64:(hh + 1) * 64,
                                   nkt * 128:(nkt + 1) * 128],
                        rhs=qt[hh * 64:(hh + 1) * 64,
                               nqb * 512:(nqb + 1) * 512],
                        start=True,
                        stop=True,
                    )
                return st

            def exp_wave(st):
                e_t = epool.tile([128, 1024], BF16, tag="e")
                nc.scalar.activation(
                    e_t[:], st[:],
                    mybir.ActivationFunctionType.Exp,
                    scale=SCALE,
                )
                return e_t

            def u_wave(u_pair, w, e_t):
                pair = BLOCKS[w // NKT][1]
                nkt = w % NKT
                for hh, u_t in ((0, u_pair[0]), (1, u_pair[1])):
                    h = pair * 2 + hh
                    nc.tensor.matmul(
                        u_t[0:65, :],
                        lhsT=v_sb[:, nkt, h * 65:h * 65 + 65],
                        rhs=e_t[:, hh * 512:(hh + 1) * 512],
                        start=(nkt == 0),
                        stop=(nkt == NKT - 1),
                    )

            def norm_block(b, u_pair):
                """Normalize a block's U pair into o_sb.  U is copied out of
                PSUM (bank turnover for block b+2); the last block's 1/D is
                broadcast with a PE matmul against ones (tail critical
                path), earlier blocks bounce 1/D through DRAM."""
                nqb, pair = BLOCKS[b]
                late = b >= 7
                uc_a = npool.tile([65, 512], F32, tag="uc_a")
                uc_b = npool.tile([65, 512], F32, tag="uc_b")
                nc.vector.tensor_copy(uc_a[:], u_pair[0][0:65, :])
                nc.vector.tensor_copy(uc_b[:], u_pair[1][0:65, :])
                u_a, u_b = uc_a, uc_b
                # both denominator rows into one tile so one reciprocal
                # covers them (partition offsets must be 32-aligned and a
                # span from offset 32 may not exceed 32 partitions).  The
                # first copy fills rows 0-31 with harmless v-values so
                # every reciprocal input byte is initialized.
                dn = npool.tile([33, 512], F32, tag="dn", bufs=1)
                rec = npool.tile([33, 512], BF16 if late else F32,
                                 tag="recl" if late else "rec", bufs=1)
                nc.vector.tensor_copy(dn[0:32, :], u_a[32:64, :])
                nc.vector.tensor_copy(dn[32:33, :], u_a[64:65, :])
                nc.vector.tensor_copy(dn[0:1, :], u_b[64:65, :])
                if late:
                    # bf16 reciprocal feeds the PE broadcast (ones is bf16;
                    # a per-query 0.4% scale error on 1/8 of the output)
                    with nc.allow_low_precision(
                            reason="bf16 1/D broadcast for the last block"):
                        nc.vector.reciprocal(rec[:], dn[:])
                    pr_a = psU.tile([128, 512], F32, tag="s2", name="pr_a")
                    pr_b = psU.tile([128, 512], F32, tag="s3", name="pr_b")
                    nc.tensor.matmul(pr_a[0:64, :], lhsT=ones_sb[32:33, :],
                                     rhs=rec[32:33, :], start=True, stop=True)
                    nc.tensor.matmul(pr_b[0:64, :], lhsT=ones_sb[0:1, :],
                                     rhs=rec[0:1, :], start=True, stop=True)
                    rr_a, rr_b = pr_a[0:64, :], pr_b[0:64, :]
                else:
                    nc.vector.reciprocal(rec[:], dn[:])
                    rr_a = npool.tile([64, 512], F32, tag="rr_a", bufs=1)
                    rr_b = npool.tile([64, 512], F32, tag="rr_b", bufs=1)
                    r_dr = rdram.tile([2, 512], F32, tag="rdr")
                    nc.sync.dma_start(out=r_dr[0:1, :], in_=rec[32:33, :])
                    nc.sync.dma_start(out=r_dr[1:2, :], in_=rec[0:1, :])
                    nc.sync.dma_start(
                        out=rr_a[:], in_=r_dr[0:1, :].to_broadcast([64, 512]))
                    nc.sync.dma_start(
                        out=rr_b[:], in_=r_dr[1:2, :].to_broadcast([64, 512]))
                    rr_a, rr_b = rr_a[:], rr_b[:]
                nc.vector.tensor_mul(
                    o_sb[0:64, pair, nqb * 512:(nqb + 1) * 512],
                    u_a[0:64, :], rr_a,
                )
                nc.vector.tensor_mul(
                    o_sb[64:128, pair, nqb * 512:(nqb + 1) * 512],
                    u_b[0:64, :], rr_b,
                )

            # --- chain FIFO -----------------------------------------------
            # Items: (deadline_cycle, pe_cost_ns, thunk).  Drained in order
            # while the PE budget lasts; anything overdue force-drains.
            # A chain's copy_out must be emitted BEFORE its first consumer:
            # S(w) is emitted at cycle w-1, so a chain feeding S(w) needs
            # deadline <= w-2; v(nt) feeds U at cycle U_SCHED[nt].
            fifo = deque()
            slot_rot = [0]

            def u_slot_tile(name, pin=None):
                if pin is None:
                    tag = f"s{slot_rot[0] % 4}"
                else:
                    tag = f"s{pin}"
                t = psU.tile([128, 512], F32, tag=tag, name=name)
                slot_rot[0] += 1
                return t

            def q_chain(kind, ct_or_nt, nb, deadline, pin=None):
                state = {}
                mm_ns = A_NS if kind == "a" else B_NS

                def first_mm():
                    state["acc"] = u_slot_tile("acc", pin)
                    chain_mm(0)

                def chain_mm(kt):
                    acc = state["acc"]
                    if kind == "a":
                        nc.tensor.matmul(
                            acc[:],
                            lhsT=wqk_sb[:, kt, CT_COL[ct_or_nt] * 128:
                                        CT_COL[ct_or_nt] * 128 + 128],
                            rhs=xT_sb[:, kt, nb * 512:(nb + 1) * 512],
                            start=(kt == 0), stop=(kt == KT - 1),
                        )
                    else:
                        nc.tensor.matmul(
                            acc[:, 0:GC],
                            lhsT=xT_sb[:, kt,
                                       ct_or_nt * 128:(ct_or_nt + 1) * 128],
                            rhs=wv_sb[:, kt, :],
                            start=(kt == 0), stop=(kt == KT - 1),
                        )

                def copy_out():
                    acc = state["acc"]
                    if kind == "a":
                        nc.vector.tensor_copy(
                            qk_sb[:, ct_or_nt, nb * 512:(nb + 1) * 512],
                            acc[:],
                        )
                    else:
                        nc.vector.tensor_copy(
                            v_view[:, ct_or_nt, :, 0:64],
                            acc[:, 0:GC].rearrange("p (h c) -> p h c", c=64),
                        )

                fifo.append((deadline, mm_ns, first_mm))
                for kt in range(1, KT):
                    fifo.append((deadline, mm_ns,
                                 lambda kt=kt: chain_mm(kt)))
                fifo.append((deadline, 0, copy_out))

            q_chain("a", 2, 1, 2)
            q_chain("b", 0, 0, 3)
            q_chain("b", 1, 0, 5)
            q_chain("a", 2, 2, 6)
            q_chain("b", 2, 0, 8)
            q_chain("b", 3, 0, 9)
            q_chain("a", 2, 3, 10)
            q_chain("b", 4, 0, 12)
            q_chain("b", 5, 0, 13)
            q_chain("a", 0, 1, 14)
            for nt in range(6, 16):
                q_chain("b", nt, 0, 15 + (nt - 6))
            q_chain("a", 0, 2, 26)
            q_chain("a", 0, 3, 34)
            # pair-1 chains: pinned to the s2/s3 slots, which U only claims
            # at block 1 (cycle U_T0+16)
            q_chain("a", 3, 0, 40, pin=2)
            q_chain("a", 3, 1, 43, pin=3)
            q_chain("a", 1, 0, 45, pin=2)
            q_chain("a", 3, 2, 47, pin=3)
            q_chain("a", 3, 3, 49, pin=2)
            q_chain("a", 1, 1, 50, pin=3)
            q_chain("a", 1, 2, 51, pin=2)
            q_chain("a", 1, 3, 52, pin=3)

            # U emission map: waves land on even cycles in 2-cycle bursts,
            # halving the S<->U weight-switch boundaries on the PE
            u_emit = {}
            for w in range(NW):
                c = U_SCHED[w]
                u_emit.setdefault(c + (c & 1), []).append(w)

            # --- prologue --------------------------------------------------
            # HAM warm-up: harmless M=1 matmuls bridge the DMA wait
            warm_ps = psU.tile([128, 512], F32, tag="s0", name="warm_ps")
            for i in range(11):
                nc.tensor.matmul(
                    warm_ps[0:1, :],
                    lhsT=ones_sb[:, 0:1], rhs=warm_sb[:],
                    start=(i == 0), stop=(i == 10),
                )
            # only the two chains feeding S(0): k-pair0 nb0, q-pair0 nb0
            for ct, nb in ((2, 0), (0, 0)):
                accp = u_slot_tile(f"accp{ct}{nb}")
                for kt in range(KT):
                    nc.tensor.matmul(
                        accp[:],
                        lhsT=wqk_sb[:, kt, CT_COL[ct] * 128:
                                    CT_COL[ct] * 128 + 128],
                        rhs=xT_sb[:, kt, nb * 512:(nb + 1) * 512],
                        start=(kt == 0), stop=(kt == KT - 1))
                nc.vector.tensor_copy(
                    qk_sb[:, ct, nb * 512:(nb + 1) * 512], accp[:])

            u_pairs = {}

            def get_u_pair(b):
                if b not in u_pairs:
                    base = (b % 2) * 2
                    t_a = psU.tile([128, 512], F32, tag=f"s{base}",
                                   name=f"ua{b}")
                    t_b = psU.tile([128, 512], F32, tag=f"s{base + 1}",
                                   name=f"ub{b}")
                    u_pairs[b] = (t_a, t_b)
                return u_pairs[b]

            def proj_piece(mt):
                """Projection for token block mt.  Pieces rotate over four
                accumulator homes (psS tile halves / the two psU pairs) so
                four are in flight; PSUM->SBUF copies balance scalar and
                vector; DRAM stores issue from the idle gpsimd queue."""
                ot = opool.tile([128, DIM], BF16, tag="ot", name="ot")
                # pieces alternate between a psS tile (whole-piece scalar
                # copy) and a psU pair (two vector copies) for a 4-deep
                # accumulator pipeline; the psU-homed pieces' tag rotation
                # follows the last U blocks, which is tail-safe.
                acc_w = None
                if mt % 2 == 0:
                    acc_w = psS.tile([128, 1024], F32, tag="st",
                                     name=f"pj{mt}")
                    accs = [acc_w[:, 0:512], acc_w[:, 512:1024]]
                else:
                    base = 2 * ((mt // 2) % 2)
                    accs = [psU.tile([128, 512], F32, tag=f"s{base}",
                                     name=f"pj{mt}a")[:],
                            psU.tile([128, 512], F32, tag=f"s{base + 1}",
                                     name=f"pj{mt}b")[:]]
                for nh in range(2):
                    acc = accs[nh]
                    for pair in range(2):
                        nc.tensor.matmul(
                            acc,
                            lhsT=o_sb[:, pair, mt * 128:(mt + 1) * 128],
                            rhs=wp_sb[:, pair, nh * 512:(nh + 1) * 512],
                            start=(pair == 0), stop=(pair == 1),
                        )
                    if mt % 2 == 1:
                        nc.vector.tensor_copy(
                            ot[:, nh * 512:(nh + 1) * 512], acc)
                if mt % 2 == 0:
                    nc.scalar.copy(ot[:], acc_w[:])
                nc.gpsimd.dma_start(
                    out=out[mt * 128:(mt + 1) * 128, :], in_=ot[:])

            # --- main loop -------------------------------------------------
            e_tiles = {}
            st_prev = s_pair(0)
            for cyc in range(NW + 6):
                budget = 1100.0
                if cyc < NW:
                    e_tiles[cyc] = exp_wave(st_prev)
                    if cyc + 1 < NW:
                        st_prev = s_pair(cyc + 1)
                        budget -= 216
                for w in u_emit.get(cyc, []):
                    b = w // NKT
                    u_wave(get_u_pair(b), w, e_tiles[w])
                    budget -= 2 * A_NS
                    if w % NKT == NKT - 1:
                        norm_block(b, u_pairs[b])
                spent = 0.0
                while fifo and (
                    any(d is not None and d <= cyc for d, _, _ in fifo)
                    or spent + fifo[0][1] <= budget
                ):
                    _, cost, thunk = fifo.popleft()
                    thunk()
                    spent += cost
            while fifo:
                fifo.popleft()[2]()

            # --- tail: full output projection (mt 0-11 run while the last
            # block normalizes; mt 12-15 need norm_block(7))
            for mt in range(16):
                proj_piece(mt)

    _split_multi_waits(nc)
    return nc


_NC_CACHE = None


def _get_nc():
    global _NC_CACHE
    if _NC_CACHE is None:
        _NC_CACHE = build_core_kernel()
    return _NC_CACHE


def build_in_maps(x, W_qkv, W_proj):
    """Per-core input tensors for the 8-way (batch, head-group) sharding."""
    x = np.asarray(x)
    W_qkv = np.asarray(W_qkv, dtype=np.float32)
    W_proj = np.asarray(W_proj, dtype=np.float32)
    bf = ml_dtypes.bfloat16
    Q = W_qkv[:, 0:DIM]
    K = W_qkv[:, DIM:2 * DIM]
    V = W_qkv[:, 2 * DIM:3 * DIM]
    in_maps = []
    for core in range(8):
        b, g = divmod(core, 4)
        sl = slice(g * GC, (g + 1) * GC)
        Qs, Ks, Vs = Q[:, sl], K[:, sl], V[:, sl]
        in_maps.append({
            "xT": np.ascontiguousarray(x[b].T).astype(bf),
            # [k-pair0 | q-pair0 | k-pair1 | q-pair1]: the first half
            # unlocks the first chains with a half-tensor DMA
            "wqk": np.ascontiguousarray(np.concatenate(
                [Ks[:, 0:128], Qs[:, 0:128], Ks[:, 128:256], Qs[:, 128:256]],
                axis=1)).astype(bf),
            "wv": np.ascontiguousarray(Vs).astype(bf),
            "wp": np.ascontiguousarray(W_proj[sl, :]).astype(bf),
        })
    return in_maps


def kernel(x, importance_weights, W_qkv, W_proj, b_proj, persistence_bias,
           _results_hook=None):
    b_proj = np.asarray(b_proj, dtype=np.float32)
    in_maps = build_in_maps(x, W_qkv, W_proj)

    nc = _get_nc()
    res = run_bass_kernel_spmd(nc, in_maps, list(range(8)))
    if _results_hook is not None:
        _results_hook(res)

    out = np.zeros((B, N, DIM), dtype=np.float32)
    for core in range(8):
        b = core // 4
        out[b] += res.results[core]["out"].astype(np.float32)
    out += b_proj[None, None, :]
    return out
